# revision 7
# baseline (speedup 1.0000x reference)
"""Trainium2 Bass kernel for the LSTM-unit problem (B=262144, I=H=C=O=128).

Strategy (data-parallel over 8 NeuronCores, batch-sharded):
  - Host pre-transposes the big activations to [feature, batch] layout and
    converts them to fp16 (the wire format): halves HBM traffic vs f32.
  - Gate-major PSUM layout: each gate's pre-activation for a whole 2048-col
    supertile is one [128, 2048] PSUM tile (4 banks), drained by a single
    ScalarE activation with the per-partition gate bias fused.
  - 1024-col moving operands (fp16) halve the matmul + LDWEIGHTS count.
  - The y-GEMM for supertile s runs in the middle of supertile s+1's gate
    phase: it needs the full z/i/f/c/h chain of s, and running it in-line
    stalled TensorE ~9us per supertile in the previous revision.
  - zi*z runs on the otherwise-idle GpSimd engine; the rest of the c/h
    assembly is fp16 DVE (2x mode).
  - Outputs come back fp16 [128, B_shard]; host re-transposes to f32.
"""

import numpy as np

B = 262144
F = 128          # feature dim (I = H = C = O = 128)
N_CORES = 8
B_SH = B // N_CORES          # 32768 cols per core
ST = 2048                    # supertile batch columns
N_ST = B_SH // ST            # 16 supertiles
BLK = 512                    # matmul moving-operand columns
N_BLK = ST // BLK

_PROGRAM_CACHE = {}


def _build_program():
    import concourse.mybir as mybir
    import concourse.tile as tile
    from concourse import bacc

    dt = mybir.dt
    Act = mybir.ActivationFunctionType

    nc = bacc.Bacc("TRN2", debug=False, num_devices=N_CORES)

    xT = nc.declare_dram_parameter("xT", [F, B_SH], dt.float16, isOutput=False)
    hT = nc.declare_dram_parameter("hT", [F, B_SH], dt.float16, isOutput=False)
    cT = nc.declare_dram_parameter("cT", [F, B_SH], dt.float16, isOutput=False)
    cT_o = nc.declare_dram_parameter("cT_o", [F, B_SH], dt.float16, isOutput=True)
    hT_o = nc.declare_dram_parameter("hT_o", [F, B_SH], dt.float16, isOutput=True)
    yT_o = nc.declare_dram_parameter("yT_o", [F, B_SH], dt.float16, isOutput=True)

    # replicated weights (host-prepared layouts), gate-major columns [z|i|f|o]
    wx = nc.declare_dram_parameter("wx", [F, 4 * F], dt.float16, isOutput=False)
    wh = nc.declare_dram_parameter("wh", [F, 4 * F], dt.float16, isOutput=False)
    wo = nc.declare_dram_parameter("wo", [F, F], dt.float16, isOutput=False)
    bg = nc.declare_dram_parameter("bg", [F, 4], dt.float32, isOutput=False)
    bo2 = nc.declare_dram_parameter("bo2", [F, 1], dt.float32, isOutput=False)

    with tile.TileContext(nc) as tc:
        with (
            tc.tile_pool(name="wpool", bufs=1) as wpool,
            tc.tile_pool(name="io", bufs=3) as io,
            tc.tile_pool(name="oio", bufs=2) as oio,
            tc.tile_pool(name="tpool", bufs=2) as tpool,
            tc.tile_pool(name="misc", bufs=2) as misc,
            tc.tile_pool(name="ps", bufs=2, space="PSUM") as ps,
        ):
            wx_sb = wpool.tile([F, 4 * F], dt.float16, tag="wx")
            wh_sb = wpool.tile([F, 4 * F], dt.float16, tag="wh")
            wo_sb = wpool.tile([F, F], dt.float16, tag="wo")
            bg_sb = wpool.tile([F, 4], dt.float32, tag="bg")
            bo2_sb = wpool.tile([F, 1], dt.float32, tag="bo2")
            nc.sync.dma_start(wx_sb[:], wx[:])
            nc.sync.dma_start(wh_sb[:], wh[:])
            nc.sync.dma_start(wo_sb[:], wo[:])
            nc.sync.dma_start(bg_sb[:], bg[:])
            nc.sync.dma_start(bo2_sb[:], bo2[:])

            def gate_gemm_act(gi, gname, xr, hr, width):
                """GEMM + activation drain for one gate; returns the fp16 tile."""
                psg = ps.tile([F, width], dt.float32, tag="ps", name=f"ps_{gname}")
                gsl = slice(gi * F, (gi + 1) * F)
                for bk in range(width // BLK):
                    bs = slice(bk * BLK, (bk + 1) * BLK)
                    nc.tensor.matmul(
                        psg[:, bs], wx_sb[:, gsl], xr[:, bs],
                        start=True, stop=False,
                    )
                for bk in range(width // BLK):
                    bs = slice(bk * BLK, (bk + 1) * BLK)
                    nc.tensor.matmul(
                        psg[:, bs], wh_sb[:, gsl], hr[:, bs],
                        start=False, stop=True,
                    )
                gt = tpool.tile([F, width], dt.float16, tag=f"t_{gname}", name=f"t_{gname}")
                func = Act.Tanh if gname == "z" else Act.Sigmoid
                nc.scalar.activation(gt[:], psg[:], func, bias=bg_sb[:, gi:gi + 1])
                return gt

            def y_stage(prev):
                """y = sigmoid(w_out @ h + b_out) for the previous chunk."""
                off, width, hto_prev = prev
                psy = ps.tile([F, width], dt.float32, tag="ps", name="ps_y")
                for bk in range(width // BLK):
                    bs = slice(bk * BLK, (bk + 1) * BLK)
                    nc.tensor.matmul(
                        psy[:, bs], wo_sb[:], hto_prev[:, bs],
                        start=True, stop=True,
                    )
                yto = oio.tile([F, width], dt.float16, tag="yto")
                nc.scalar.activation(yto[:], psy[:], Act.Sigmoid, bias=bo2_sb[:])
                nc.gpsimd.dma_start(yT_o[:, off:off + width], yto[:])

            # first/last supertiles split in half so the pipeline fills and
            # drains in half the time (ramp was ~18us, tail ~8us at ST=2048)
            chunks = [(0, ST // 2), (ST // 2, ST // 2)]
            chunks += [(s * ST, ST) for s in range(1, N_ST - 1)]
            chunks += [((N_ST - 1) * ST, ST // 2), ((N_ST - 1) * ST + ST // 2, ST // 2)]

            prev = None
            for off, width in chunks:
                ss = slice(off, off + width)
                xr = io.tile([F, width], dt.float16, tag="xr")
                hr = io.tile([F, width], dt.float16, tag="hr")
                ci = io.tile([F, width], dt.float16, tag="ci")
                nc.sync.dma_start(xr[:], xT[:, ss])
                nc.sync.dma_start(hr[:], hT[:, ss])
                nc.sync.dma_start(ci[:], cT[:, ss])

                tz = gate_gemm_act(0, "z", xr, hr, width)
                ti = gate_gemm_act(1, "i", xr, hr, width)
                tf = gate_gemm_act(2, "f", xr, hr, width)
                to_ = gate_gemm_act(3, "o", xr, hr, width)

                # c = zf*c_ + zi*z
                t1 = misc.tile([F, width], dt.float16, tag="t1")
                nc.vector.tensor_mul(t1[:], ti[:], tz[:])
                cto = oio.tile([F, width], dt.float16, tag="cto")
                nc.vector.tensor_mul(cto[:], tf[:], ci[:])
                nc.vector.tensor_add(cto[:], cto[:], t1[:])

                tcn = misc.tile([F, width], dt.float16, tag="tcn")
                nc.scalar.activation(tcn[:], cto[:], Act.Tanh)

                # y(prev) after tanh(c): its GEMM (PSUM slot freed by f-act)
                # finishes while ScalarE runs o-act + tanh_c
                if prev is not None:
                    y_stage(prev)
                else:
                    ps.tile([F, width], dt.float32, tag="ps", name="ps_pad0")

                hto = oio.tile([F, width], dt.float16, tag="hto")
                nc.vector.tensor_mul(hto[:], to_[:], tcn[:])

                nc.gpsimd.dma_start(cT_o[:, ss], cto[:])
                nc.gpsimd.dma_start(hT_o[:, ss], hto[:])
                prev = (off, width, hto)

                # 6th allocation keeps the 2-buf PSUM ring parity constant:
                # with [z,i,f,o,y,pad] the next iteration's z waits on this
                # never-written pad (instant) and i waits on y(prev)-act
                ps.tile([F, width], dt.float32, tag="ps", name="ps_pad")

            y_stage(prev)

    nc.finalize()
    return nc


def kernel(c_, h_, x, w, wi, wf, wo, w_out, b, bi, bf, bo, b_out):
    from concourse.bass_utils import run_bass_kernel_spmd

    if "nc" not in _PROGRAM_CACHE:
        _PROGRAM_CACHE["nc"] = _build_program()
    nc = _PROGRAM_CACHE["nc"]

    c_ = np.asarray(c_, dtype=np.float32)
    h_ = np.asarray(h_, dtype=np.float32)
    x = np.asarray(x, dtype=np.float32)

    # host weight prep: W_stack rows ordered [z, i, f, o]
    W_stack = np.concatenate(
        [np.asarray(a, np.float32) for a in (w, wi, wf, wo)], axis=0
    )  # [512, 256]
    wx_h = np.ascontiguousarray(W_stack[:, :F].T.astype(np.float16))   # [128, 512]
    wh_h = np.ascontiguousarray(W_stack[:, F:].T.astype(np.float16))   # [128, 512]
    wo_h = np.ascontiguousarray(np.asarray(w_out, np.float32).T.astype(np.float16))
    bg_h = np.ascontiguousarray(
        np.stack(
            [np.asarray(v, np.float32) for v in (b, bi, bf, bo)], axis=1
        )
    )  # [128, 4]
    bo2_h = np.ascontiguousarray(np.asarray(b_out, np.float32).reshape(F, 1))

    xs = x.reshape(N_CORES, B_SH, F)
    hs = h_.reshape(N_CORES, B_SH, F)
    cs = c_.reshape(N_CORES, B_SH, F)
    in_maps = []
    for i in range(N_CORES):
        in_maps.append(
            {
                "xT": np.ascontiguousarray(xs[i].T.astype(np.float16)),
                "hT": np.ascontiguousarray(hs[i].T.astype(np.float16)),
                "cT": np.ascontiguousarray(cs[i].T.astype(np.float16)),
                "wx": wx_h,
                "wh": wh_h,
                "wo": wo_h,
                "bg": bg_h,
                "bo2": bo2_h,
            }
        )

    _PROGRAM_CACHE["in_maps"] = in_maps
    res = run_bass_kernel_spmd(nc, in_maps, list(range(N_CORES)))

    c_out = np.empty((B, F), np.float32)
    h_out = np.empty((B, F), np.float32)
    y_out = np.empty((B, F), np.float32)
    for i in range(N_CORES):
        r = res.results[i]
        sl = slice(i * B_SH, (i + 1) * B_SH)
        c_out[sl] = r["cT_o"].T.astype(np.float32)
        h_out[sl] = r["hT_o"].T.astype(np.float32)
        y_out[sl] = r["yT_o"].T.astype(np.float32)
    return (c_out, h_out, y_out)


# revision 10
# speedup vs baseline: 1.1796x; 1.1796x over previous
"""Trainium2 Bass kernel for the LSTM-unit problem (B=262144, I=H=C=O=128).

Strategy (data-parallel over 8 NeuronCores, batch-sharded):
  - Host pre-transposes the big activations to [feature, batch] layout and
    converts them to fp16 (the wire format): halves HBM traffic vs f32.
  - Gate-major PSUM layout: each gate's pre-activation for a whole 2048-col
    supertile is one [128, 2048] PSUM tile (4 banks), drained by a single
    ScalarE activation with the per-partition gate bias fused.
  - 1024-col moving operands (fp16) halve the matmul + LDWEIGHTS count.
  - The y-GEMM for supertile s runs in the middle of supertile s+1's gate
    phase: it needs the full z/i/f/c/h chain of s, and running it in-line
    stalled TensorE ~9us per supertile in the previous revision.
  - zi*z runs on the otherwise-idle GpSimd engine; the rest of the c/h
    assembly is fp16 DVE (2x mode).
  - Outputs come back fp16 [128, B_shard]; host re-transposes to f32.
"""

import numpy as np

B = 262144
F = 128          # feature dim (I = H = C = O = 128)
N_CORES = 8
B_SH = B // N_CORES          # 32768 cols per core
ST = 2048                    # supertile batch columns
N_ST = B_SH // ST            # 16 supertiles
BLK = 512                    # matmul moving-operand columns
N_BLK = ST // BLK

_PROGRAM_CACHE = {}


def _build_program():
    import concourse.mybir as mybir
    import concourse.tile as tile
    from concourse import bacc

    dt = mybir.dt
    Act = mybir.ActivationFunctionType

    nc = bacc.Bacc("TRN2", debug=False, num_devices=N_CORES)

    xT = nc.declare_dram_parameter("xT", [F, B_SH], dt.float16, isOutput=False)
    hT = nc.declare_dram_parameter("hT", [F, B_SH], dt.float16, isOutput=False)
    cT = nc.declare_dram_parameter("cT", [F, B_SH], dt.float16, isOutput=False)
    cT_o = nc.declare_dram_parameter("cT_o", [F, B_SH], dt.float16, isOutput=True)
    hT_o = nc.declare_dram_parameter("hT_o", [F, B_SH], dt.float16, isOutput=True)
    yT_o = nc.declare_dram_parameter("yT_o", [F, B_SH], dt.float16, isOutput=True)

    # replicated weights (host-prepared layouts), gate-major columns [z|i|f|o]
    wx = nc.declare_dram_parameter("wx", [F, 4 * F], dt.float16, isOutput=False)
    wh = nc.declare_dram_parameter("wh", [F, 4 * F], dt.float16, isOutput=False)
    wo = nc.declare_dram_parameter("wo", [F, F], dt.float16, isOutput=False)
    bg = nc.declare_dram_parameter("bg", [F, 4], dt.float32, isOutput=False)
    bo2 = nc.declare_dram_parameter("bo2", [F, 1], dt.float32, isOutput=False)

    with tile.TileContext(nc) as tc:
        with (
            tc.tile_pool(name="wpool", bufs=1) as wpool,
            tc.tile_pool(name="io", bufs=3) as io,
            tc.tile_pool(name="oio", bufs=2) as oio,
            tc.tile_pool(name="tpool", bufs=2) as tpool,
            tc.tile_pool(name="misc", bufs=2) as misc,
            tc.tile_pool(name="ps", bufs=2, space="PSUM") as ps,
        ):
            wx_sb = wpool.tile([F, 4 * F], dt.float16, tag="wx")
            wh_sb = wpool.tile([F, 4 * F], dt.float16, tag="wh")
            wo_sb = wpool.tile([F, F], dt.float16, tag="wo")
            bg_sb = wpool.tile([F, 4], dt.float32, tag="bg")
            bo2_sb = wpool.tile([F, 1], dt.float32, tag="bo2")

            def gate_gemm_act(gi, gname, xr, hr, interleave=False, halve=False):
                """GEMM + activation drain for one gate; returns the fp16 tile.

                interleave: finish PSUM blocks in column order (x then h per
                block) so a halved activation can drain the first half while
                the second is still being computed -- used only on the ramp
                iteration, where ScalarE would otherwise idle.
                """
                psg = ps.tile([F, ST], dt.float32, tag="ps", name=f"ps_{gname}")
                gsl = slice(gi * F, (gi + 1) * F)
                if interleave:
                    for bk in range(N_BLK):
                        bs = slice(bk * BLK, (bk + 1) * BLK)
                        nc.tensor.matmul(
                            psg[:, bs], wx_sb[:, gsl], xr[:, bs],
                            start=True, stop=False,
                        )
                        nc.tensor.matmul(
                            psg[:, bs], wh_sb[:, gsl], hr[:, bs],
                            start=False, stop=True,
                        )
                else:
                    for bk in range(N_BLK):
                        bs = slice(bk * BLK, (bk + 1) * BLK)
                        nc.tensor.matmul(
                            psg[:, bs], wx_sb[:, gsl], xr[:, bs],
                            start=True, stop=False,
                        )
                    for bk in range(N_BLK):
                        bs = slice(bk * BLK, (bk + 1) * BLK)
                        nc.tensor.matmul(
                            psg[:, bs], wh_sb[:, gsl], hr[:, bs],
                            start=False, stop=True,
                        )
                gt = tpool.tile([F, ST], dt.float16, tag=f"t_{gname}", name=f"t_{gname}")
                func = Act.Tanh if gname == "z" else Act.Sigmoid
                if halve:
                    hw = ST // 2
                    nc.scalar.activation(gt[:, :hw], psg[:, :hw], func,
                                         bias=bg_sb[:, gi:gi + 1])
                    nc.scalar.activation(gt[:, hw:], psg[:, hw:], func,
                                         bias=bg_sb[:, gi:gi + 1])
                else:
                    nc.scalar.activation(gt[:], psg[:], func, bias=bg_sb[:, gi:gi + 1])
                return gt

            def y_stage(s_prev, hto_prev):
                """y = sigmoid(w_out @ h + b_out) for the previous supertile."""
                ss = slice(s_prev * ST, (s_prev + 1) * ST)
                psy = ps.tile([F, ST], dt.float32, tag="ps", name="ps_y")
                for bk in range(N_BLK):
                    bs = slice(bk * BLK, (bk + 1) * BLK)
                    nc.tensor.matmul(
                        psy[:, bs], wo_sb[:], hto_prev[:, bs],
                        start=True, stop=True,
                    )
                yto = oio.tile([F, ST], dt.float16, tag="yto")
                nc.scalar.activation(yto[:], psy[:], Act.Sigmoid, bias=bo2_sb[:])
                nc.gpsimd.dma_start(yT_o[:, ss], yto[:])

            hto_prev = None
            for s in range(N_ST):
                ss = slice(s * ST, (s + 1) * ST)
                xr = io.tile([F, ST], dt.float16, tag="xr")
                hr = io.tile([F, ST], dt.float16, tag="hr")
                ci = io.tile([F, ST], dt.float16, tag="ci")
                if s == 0:
                    # ramp: land the first half of x/h plus the gate weights
                    # before anything else so the first GEMM+act start ASAP
                    hw = ST // 2
                    nc.sync.dma_start(xr[:, :hw], xT[:, :hw])
                    nc.sync.dma_start(hr[:, :hw], hT[:, :hw])
                    nc.sync.dma_start(wx_sb[:], wx[:])
                    nc.sync.dma_start(wh_sb[:], wh[:])
                    nc.sync.dma_start(bg_sb[:], bg[:])
                    nc.sync.dma_start(xr[:, hw:], xT[:, hw:ST])
                    nc.sync.dma_start(hr[:, hw:], hT[:, hw:ST])
                    nc.sync.dma_start(ci[:], cT[:, ss])
                    nc.sync.dma_start(wo_sb[:], wo[:])
                    nc.sync.dma_start(bo2_sb[:], bo2[:])
                else:
                    nc.sync.dma_start(xr[:], xT[:, ss])
                    nc.sync.dma_start(hr[:], hT[:, ss])
                    nc.sync.dma_start(ci[:], cT[:, ss])

                ramp = s == 0
                tz = gate_gemm_act(0, "z", xr, hr, interleave=ramp, halve=ramp)
                ti = gate_gemm_act(1, "i", xr, hr, interleave=ramp, halve=ramp)
                tf = gate_gemm_act(2, "f", xr, hr)
                to_ = gate_gemm_act(3, "o", xr, hr)

                # c = zf*c_ + zi*z
                t1 = misc.tile([F, ST], dt.float16, tag="t1")
                nc.vector.tensor_mul(t1[:], ti[:], tz[:])
                cto = oio.tile([F, ST], dt.float16, tag="cto")
                nc.vector.tensor_mul(cto[:], tf[:], ci[:])
                nc.vector.tensor_add(cto[:], cto[:], t1[:])

                tcn = misc.tile([F, ST], dt.float16, tag="tcn")
                nc.scalar.activation(tcn[:], cto[:], Act.Tanh)

                # y(s-1) after tanh(c): its GEMM (PSUM slot freed by f-act)
                # finishes while ScalarE runs o-act + tanh_c
                if hto_prev is not None:
                    y_stage(s - 1, hto_prev)
                else:
                    ps.tile([F, ST], dt.float32, tag="ps", name="ps_pad0")

                hto = oio.tile([F, ST], dt.float16, tag="hto")
                nc.vector.tensor_mul(hto[:], to_[:], tcn[:])

                nc.gpsimd.dma_start(cT_o[:, ss], cto[:])
                nc.gpsimd.dma_start(hT_o[:, ss], hto[:])
                hto_prev = hto

                # 6th allocation keeps the 2-buf PSUM ring parity constant:
                # with [z,i,f,o,y,pad] the next iteration's z waits on this
                # never-written pad (instant) and i waits on y(s-1)-act
                ps.tile([F, ST], dt.float32, tag="ps", name="ps_pad")

            y_stage(N_ST - 1, hto_prev)

    nc.finalize()
    return nc


def kernel(c_, h_, x, w, wi, wf, wo, w_out, b, bi, bf, bo, b_out):
    from concourse.bass_utils import run_bass_kernel_spmd

    if "nc" not in _PROGRAM_CACHE:
        _PROGRAM_CACHE["nc"] = _build_program()
    nc = _PROGRAM_CACHE["nc"]

    c_ = np.asarray(c_, dtype=np.float32)
    h_ = np.asarray(h_, dtype=np.float32)
    x = np.asarray(x, dtype=np.float32)

    # host weight prep: W_stack rows ordered [z, i, f, o]
    W_stack = np.concatenate(
        [np.asarray(a, np.float32) for a in (w, wi, wf, wo)], axis=0
    )  # [512, 256]
    wx_h = np.ascontiguousarray(W_stack[:, :F].T.astype(np.float16))   # [128, 512]
    wh_h = np.ascontiguousarray(W_stack[:, F:].T.astype(np.float16))   # [128, 512]
    wo_h = np.ascontiguousarray(np.asarray(w_out, np.float32).T.astype(np.float16))
    bg_h = np.ascontiguousarray(
        np.stack(
            [np.asarray(v, np.float32) for v in (b, bi, bf, bo)], axis=1
        )
    )  # [128, 4]
    bo2_h = np.ascontiguousarray(np.asarray(b_out, np.float32).reshape(F, 1))

    xs = x.reshape(N_CORES, B_SH, F)
    hs = h_.reshape(N_CORES, B_SH, F)
    cs = c_.reshape(N_CORES, B_SH, F)
    in_maps = []
    for i in range(N_CORES):
        in_maps.append(
            {
                "xT": np.ascontiguousarray(xs[i].T.astype(np.float16)),
                "hT": np.ascontiguousarray(hs[i].T.astype(np.float16)),
                "cT": np.ascontiguousarray(cs[i].T.astype(np.float16)),
                "wx": wx_h,
                "wh": wh_h,
                "wo": wo_h,
                "bg": bg_h,
                "bo2": bo2_h,
            }
        )

    _PROGRAM_CACHE["in_maps"] = in_maps
    res = run_bass_kernel_spmd(nc, in_maps, list(range(N_CORES)))

    c_out = np.empty((B, F), np.float32)
    h_out = np.empty((B, F), np.float32)
    y_out = np.empty((B, F), np.float32)
    for i in range(N_CORES):
        r = res.results[i]
        sl = slice(i * B_SH, (i + 1) * B_SH)
        c_out[sl] = r["cT_o"].T.astype(np.float32)
        h_out[sl] = r["hT_o"].T.astype(np.float32)
        y_out[sl] = r["yT_o"].T.astype(np.float32)
    return (c_out, h_out, y_out)


# revision 11
# speedup vs baseline: 1.1943x; 1.0125x over previous
"""Trainium2 Bass kernel for the LSTM-unit problem (B=262144, I=H=C=O=128).

Strategy (data-parallel over 8 NeuronCores, batch-sharded):
  - Host pre-transposes the big activations to [feature, batch] layout and
    converts them to fp16 (the wire format): halves HBM traffic vs f32.
  - Gate-major PSUM layout: each gate's pre-activation for a whole 2048-col
    supertile is one [128, 2048] PSUM tile (4 banks), drained by a single
    ScalarE activation with the per-partition gate bias fused.
  - The y-GEMM for supertile s runs during supertile s+1's gate phase.
  - zi*z and the rest of the c/h assembly are fp16 DVE (2x mode).
  - Outputs come back fp16 [128, B_shard]; host re-transposes to f32.
"""

import numpy as np

B = 262144
F = 128          # feature dim (I = H = C = O = 128)
N_CORES = 8
B_SH = B // N_CORES          # 32768 cols per core
ST = 2048                    # supertile batch columns
N_ST = B_SH // ST            # 16 supertiles
BLK = 512                    # matmul moving-operand columns
N_BLK = ST // BLK

_PROGRAM_CACHE = {}


def _build_program():
    import concourse.mybir as mybir
    import concourse.tile as tile
    from concourse import bacc

    dt = mybir.dt
    Act = mybir.ActivationFunctionType

    nc = bacc.Bacc("TRN2", debug=False, num_devices=N_CORES)

    xT = nc.declare_dram_parameter("xT", [F, B_SH], dt.float16, isOutput=False)
    hT = nc.declare_dram_parameter("hT", [F, B_SH], dt.float16, isOutput=False)
    cT = nc.declare_dram_parameter("cT", [F, B_SH], dt.float16, isOutput=False)
    cT_o = nc.declare_dram_parameter("cT_o", [F, B_SH], dt.float16, isOutput=True)
    hT_o = nc.declare_dram_parameter("hT_o", [F, B_SH], dt.float16, isOutput=True)
    yT_o = nc.declare_dram_parameter("yT_o", [F, B_SH], dt.float16, isOutput=True)

    # replicated weights (host-prepared layouts), gate-major columns [z|i|f|o]
    wx = nc.declare_dram_parameter("wx", [F, 4 * F], dt.float16, isOutput=False)
    wh = nc.declare_dram_parameter("wh", [F, 4 * F], dt.float16, isOutput=False)
    wo = nc.declare_dram_parameter("wo", [F, F], dt.float16, isOutput=False)
    bg = nc.declare_dram_parameter("bg", [F, 4], dt.float32, isOutput=False)
    bo2 = nc.declare_dram_parameter("bo2", [F, 1], dt.float32, isOutput=False)

    with tile.TileContext(nc) as tc:
        with (
            tc.tile_pool(name="wpool", bufs=1) as wpool,
            tc.tile_pool(name="io", bufs=3) as io,
            tc.tile_pool(name="oio", bufs=2) as oio,
            tc.tile_pool(name="tpool", bufs=2) as tpool,
            tc.tile_pool(name="misc", bufs=2) as misc,
            tc.tile_pool(name="ps", bufs=2, space="PSUM") as ps,
        ):
            wx_sb = wpool.tile([F, 4 * F], dt.float16, tag="wx")
            wh_sb = wpool.tile([F, 4 * F], dt.float16, tag="wh")
            wo_sb = wpool.tile([F, F], dt.float16, tag="wo")
            bg_sb = wpool.tile([F, 4], dt.float32, tag="bg")
            bo2_sb = wpool.tile([F, 1], dt.float32, tag="bo2")
            nc.sync.dma_start(wx_sb[:], wx[:])
            nc.sync.dma_start(wh_sb[:], wh[:])
            nc.sync.dma_start(wo_sb[:], wo[:])
            nc.sync.dma_start(bg_sb[:], bg[:])
            nc.sync.dma_start(bo2_sb[:], bo2[:])

            def gate_gemm_act(gi, gname, xr, hr):
                """GEMM + activation drain for one gate; returns the fp16 tile."""
                psg = ps.tile([F, ST], dt.float32, tag="ps", name=f"ps_{gname}")
                gsl = slice(gi * F, (gi + 1) * F)
                for bk in range(N_BLK):
                    bs = slice(bk * BLK, (bk + 1) * BLK)
                    nc.tensor.matmul(
                        psg[:, bs], wx_sb[:, gsl], xr[:, bs],
                        start=True, stop=False,
                    )
                for bk in range(N_BLK):
                    bs = slice(bk * BLK, (bk + 1) * BLK)
                    nc.tensor.matmul(
                        psg[:, bs], wh_sb[:, gsl], hr[:, bs],
                        start=False, stop=True,
                    )
                gt = tpool.tile([F, ST], dt.float16, tag=f"t_{gname}", name=f"t_{gname}")
                func = Act.Tanh if gname == "z" else Act.Sigmoid
                nc.scalar.activation(gt[:], psg[:], func, bias=bg_sb[:, gi:gi + 1])
                return gt

            def y_stage(s_prev, hto_prev):
                """y = sigmoid(w_out @ h + b_out) for the previous supertile."""
                ss = slice(s_prev * ST, (s_prev + 1) * ST)
                psy = ps.tile([F, ST], dt.float32, tag="ps", name="ps_y")
                for bk in range(N_BLK):
                    bs = slice(bk * BLK, (bk + 1) * BLK)
                    nc.tensor.matmul(
                        psy[:, bs], wo_sb[:], hto_prev[:, bs],
                        start=True, stop=True,
                    )
                yto = oio.tile([F, ST], dt.float16, tag="yto")
                nc.scalar.activation(yto[:], psy[:], Act.Sigmoid, bias=bo2_sb[:])
                nc.gpsimd.dma_start(yT_o[:, ss], yto[:])

            hto_prev = None
            for s in range(N_ST):
                ss = slice(s * ST, (s + 1) * ST)
                xr = io.tile([F, ST], dt.float16, tag="xr")
                hr = io.tile([F, ST], dt.float16, tag="hr")
                ci = io.tile([F, ST], dt.float16, tag="ci")
                nc.sync.dma_start(xr[:], xT[:, ss])
                nc.sync.dma_start(hr[:], hT[:, ss])
                nc.sync.dma_start(ci[:], cT[:, ss])

                tz = gate_gemm_act(0, "z", xr, hr)
                ti = gate_gemm_act(1, "i", xr, hr)
                tf = gate_gemm_act(2, "f", xr, hr)
                to_ = gate_gemm_act(3, "o", xr, hr)

                # c = zf*c_ + zi*z
                t1 = misc.tile([F, ST], dt.float16, tag="t1")
                nc.vector.tensor_mul(t1[:], ti[:], tz[:])
                cto = oio.tile([F, ST], dt.float16, tag="cto")
                nc.vector.tensor_mul(cto[:], tf[:], ci[:])
                nc.vector.tensor_add(cto[:], cto[:], t1[:])

                tcn = misc.tile([F, ST], dt.float16, tag="tcn")
                nc.scalar.activation(tcn[:], cto[:], Act.Tanh)

                # y(s-1) after tanh(c): its GEMM (PSUM slot freed by f-act)
                # finishes while ScalarE runs o-act + tanh_c
                if hto_prev is not None:
                    y_stage(s - 1, hto_prev)
                else:
                    ps.tile([F, ST], dt.float32, tag="ps", name="ps_pad0")

                hto = oio.tile([F, ST], dt.float16, tag="hto")
                nc.vector.tensor_mul(hto[:], to_[:], tcn[:])

                nc.gpsimd.dma_start(cT_o[:, ss], cto[:])
                nc.gpsimd.dma_start(hT_o[:, ss], hto[:])
                hto_prev = hto

                # 6th allocation keeps the 2-buf PSUM ring parity constant:
                # with [z,i,f,o,y,pad] the next iteration's z waits on this
                # never-written pad (instant) and i waits on y(s-1)-act
                ps.tile([F, ST], dt.float32, tag="ps", name="ps_pad")

            y_stage(N_ST - 1, hto_prev)

    nc.finalize()
    return nc


def kernel(c_, h_, x, w, wi, wf, wo, w_out, b, bi, bf, bo, b_out):
    from concourse.bass_utils import run_bass_kernel_spmd

    if "nc" not in _PROGRAM_CACHE:
        _PROGRAM_CACHE["nc"] = _build_program()
    nc = _PROGRAM_CACHE["nc"]

    c_ = np.asarray(c_, dtype=np.float32)
    h_ = np.asarray(h_, dtype=np.float32)
    x = np.asarray(x, dtype=np.float32)

    # host weight prep: W_stack rows ordered [z, i, f, o]
    W_stack = np.concatenate(
        [np.asarray(a, np.float32) for a in (w, wi, wf, wo)], axis=0
    )  # [512, 256]
    wx_h = np.ascontiguousarray(W_stack[:, :F].T.astype(np.float16))   # [128, 512]
    wh_h = np.ascontiguousarray(W_stack[:, F:].T.astype(np.float16))   # [128, 512]
    wo_h = np.ascontiguousarray(np.asarray(w_out, np.float32).T.astype(np.float16))
    bg_h = np.ascontiguousarray(
        np.stack(
            [np.asarray(v, np.float32) for v in (b, bi, bf, bo)], axis=1
        )
    )  # [128, 4]
    bo2_h = np.ascontiguousarray(np.asarray(b_out, np.float32).reshape(F, 1))

    xs = x.reshape(N_CORES, B_SH, F)
    hs = h_.reshape(N_CORES, B_SH, F)
    cs = c_.reshape(N_CORES, B_SH, F)
    in_maps = []
    for i in range(N_CORES):
        in_maps.append(
            {
                "xT": np.ascontiguousarray(xs[i].T.astype(np.float16)),
                "hT": np.ascontiguousarray(hs[i].T.astype(np.float16)),
                "cT": np.ascontiguousarray(cs[i].T.astype(np.float16)),
                "wx": wx_h,
                "wh": wh_h,
                "wo": wo_h,
                "bg": bg_h,
                "bo2": bo2_h,
            }
        )

    _PROGRAM_CACHE["in_maps"] = in_maps
    res = run_bass_kernel_spmd(nc, in_maps, list(range(N_CORES)))

    c_out = np.empty((B, F), np.float32)
    h_out = np.empty((B, F), np.float32)
    y_out = np.empty((B, F), np.float32)
    for i in range(N_CORES):
        r = res.results[i]
        sl = slice(i * B_SH, (i + 1) * B_SH)
        c_out[sl] = r["cT_o"].T.astype(np.float32)
        h_out[sl] = r["hT_o"].T.astype(np.float32)
        y_out[sl] = r["yT_o"].T.astype(np.float32)
    return (c_out, h_out, y_out)
